# revision 17
# baseline (speedup 1.0000x reference)
# kernel.py — Bidirectional masked-GRU-with-predictor on 8 Trainium2 NeuronCores.
#
# Problem (reference.py): B=128, T=1024, H=512
#   per step, per direction:
#     x_in = where(mask, predictor(h), x)            predictor: Linear(H,H)->ReLU->Linear(H,1)->Tanh
#     h    = GRUCell(h, x_in)                        PyTorch gate order (r, z, n)
#   output [B, T, 2H] = concat(fwd hidden states, time-reversed bwd hidden states)
#
# Sharding: 8 cores = 2 directions x 4 batch groups of 32.  All cores run the
# SAME Bass program; per-core data differs (bwd cores get time-reversed x/mask
# and their outputs are flipped back on the host).
#
# On-core layout ("feature-major, chunk-in-free"):
#   h^T kept as [128 partitions = feature%128, (j,b)] where j = feature//128
#   (4 chunks), b = local batch (32).  Stationary = W^T 128x128 blocks.
#   ALL stationaries are zero-padded to [128,128] so the PE's background
#   weight loading never re-configures (mixed tile shapes cost ~100-160ns per
#   switch).  PSUM: one bank per live accumulation region (start=True clears
#   has_written for the WHOLE bank; PE-write + DVE/ACT-read of one bank is a
#   fatal collision), double-buffered by step parity: G_r, G_z, G_n, and a
#   shared PH->PRD->GIN bank, x2 = all 8 banks.  fp16 matmuls + fp32 PSUM.

import numpy as np

B, T, H = 128, 1024, 512
NCORES = 8
BL = B // 4          # 32: batch per core (4 groups x 2 directions)
KC = H // 128        # 4 contraction chunks
MC = (3 * H + H) // 128  # 16 output chunks (w_hh 12 + p_w1 4)
U_DEF = 32           # time steps per For_i iteration
WSCALE = 256.0       # stationary pre-scale (exact power of two; exact in fp16)

_cache = {}


def _build_program(t_steps=T, u_steps=U_DEF, bl=BL, n_cores=NCORES):
    import concourse.bacc as bacc
    import concourse.bass as bass
    import concourse.tile as tile
    from concourse.tile import add_dep_helper
    from concourse import mybir

    f16 = mybir.dt.float16
    f32 = mybir.dt.float32

    nc = bacc.Bacc(
        "TRN2",
        target_bir_lowering=False,
        debug=False,
        enable_asserts=False,
        num_devices=n_cores,
    )

    # ---- DRAM tensors (per-core data; same names on every core) ----
    d_wt = nc.dram_tensor("wt", [128, MC * KC * 128], f16,
                          kind="ExternalInput").ap()
    d_gi = nc.dram_tensor("gil", [128, 12 * 128], f16,
                          kind="ExternalInput").ap()
    d_bc = nc.dram_tensor("bc", [128, 5 * 128], f16,
                          kind="ExternalInput").ap()
    d_a = nc.dram_tensor("a_arr", [t_steps, bl], f16,
                         kind="ExternalInput").ap()
    d_m = nc.dram_tensor("m_arr", [t_steps, bl], f16,
                         kind="ExternalInput").ap()
    d_e4 = nc.dram_tensor("e4", [128, KC * bl], f16, kind="ExternalInput").ap()
    d_pw2 = nc.dram_tensor("pw2t", [128, KC * 128], f16,
                           kind="ExternalInput").ap()
    d_pb2 = nc.dram_tensor("pb2", [1, 1], f32, kind="ExternalInput").ap()
    d_out = nc.dram_tensor("outl", [t_steps, 128, KC, bl], f16,
                           kind="ExternalOutput").ap()

    Tanh = mybir.ActivationFunctionType.Tanh
    Sigmoid = mybir.ActivationFunctionType.Sigmoid
    SC = 1.0 / WSCALE

    with tile.TileContext(nc) as tc:
        import contextlib

        with contextlib.ExitStack() as ctx:
            consts = ctx.enter_context(tc.tile_pool(name="consts", bufs=1))
            psum = ctx.enter_context(tc.tile_pool(name="psum", bufs=1, space="PSUM"))
            work = ctx.enter_context(tc.tile_pool(name="work", bufs=2))
            io = ctx.enter_context(tc.tile_pool(name="io", bufs=2))

            # ---- constant preload ----
            WT = consts.tile([128, MC * KC * 128], f16, tag="WT")
            GIL = consts.tile([128, 12 * 128], f16, tag="GIL")
            BC = consts.tile([128, 5 * 128], f16, tag="BC")
            E4 = consts.tile([128, KC * bl], f16, tag="E4")
            PW2 = consts.tile([128, KC * 128], f16, tag="PW2")
            PB2 = consts.tile([1, 1], f32, tag="PB2")
            for dst, src in (
                (WT, d_wt), (GIL, d_gi), (BC, d_bc),
                (E4, d_e4), (PW2, d_pw2), (PB2, d_pb2),
            ):
                nc.sync.dma_start(out=dst, in_=src)

            # persistent ping-pong hidden state, fp16, [128, (j,b)]
            h0 = consts.tile([128, KC * bl], f16, tag="h0")
            h1 = consts.tile([128, KC * bl], f16, tag="h1")
            nc.vector.memset(h0, 0.0)
            nc.vector.memset(h1, 0.0)
            h_tiles = [h0, h1]

            # PSUM: G_r, G_z, G_n own a bank; PH -> PRD -> GIN share the 4th
            # (serial within a step); x2 step parity = 8 banks.
            W_ = KC * bl
            def mk_psum(tagp):
                g_r = psum.tile([128, W_], f32, tag=f"G_r{tagp}",
                                name=f"G_r{tagp}")
                g_z = psum.tile([128, W_], f32, tag=f"G_z{tagp}",
                                name=f"G_z{tagp}")
                g_n = psum.tile([128, W_], f32, tag=f"G_n{tagp}",
                                name=f"G_n{tagp}")
                phb = psum.tile([128, 2 * W_ + bl], f32, tag=f"PHB{tagp}",
                                name=f"PHB{tagp}")
                return {
                    "G_r": g_r, "G_z": g_z, "G_n": g_n,
                    "PH": phb[:, 0:W_],
                    "GIN": phb[:, W_:2 * W_],
                    "PRD": phb[:, 2 * W_:2 * W_ + bl],
                }
            P = [mk_psum(f"{p}") for p in range(2)]

            def w_block(m, k):
                bi = m * KC + k
                return WT[:, bi * 128:(bi + 1) * 128]

            state = {"prev": None}

            def pe_chain(first, last):
                if state["prev"] is not None and first is not None:
                    add_dep_helper(first.ins, state["prev"].ins, sync=False)
                if last is not None:
                    state["prev"] = last

            def emit_w_region(base_m, region, h_cur, has_gi, bias_col):
                # One E4 bias matmul opens the region (the only start=True for
                # its bank this step); W matmuls accumulate.
                first = nc.tensor.matmul(
                    region, BC[:, bias_col * 128:(bias_col + 1) * 128],
                    E4, start=True, stop=False, skip_group_check=True,
                )
                last = first
                for j in range(KC):
                    m = base_m + j
                    for k in range(KC):
                        last = nc.tensor.matmul(
                            region[:, j * bl:(j + 1) * bl],
                            w_block(m, k),
                            h_cur[:, k * bl:(k + 1) * bl],
                            start=False,
                            stop=(not has_gi and k == KC - 1),
                            skip_group_check=True,
                        )
                pe_chain(first, last)

            def emit_gi(g_idx, region, gi_rhs):
                first = last = None
                for j in range(KC):
                    gj = g_idx * KC + j
                    last = nc.tensor.matmul(
                        region[:, j * bl:(j + 1) * bl],
                        GIL[:, gj * 128:(gj + 1) * 128],
                        gi_rhs,
                        start=False, stop=True, skip_group_check=True,
                    )
                    if first is None:
                        first = last
                pe_chain(first, last)

            def step(u, S2, MB, t_dyn):
                h_cur = h_tiles[u % 2]
                h_new = h_tiles[(u + 1) % 2]
                R = P[u % 2]
                gi_rhs = S2[:, u * bl:(u + 1) * bl]

                # --- PE order: PH, W_r, PRD, W_z, gi_r, gi_z, W_n, GIN ---
                emit_w_region(12, R["PH"], h_cur, has_gi=False, bias_col=3)

                relu = work.tile([128, KC * bl], f16, tag="relu")
                nc.vector.tensor_scalar_max(relu, R["PH"], 0.0)

                emit_w_region(0, R["G_r"], h_cur, has_gi=True, bias_col=0)

                prd_f = prd_l = None
                for k in range(KC):
                    prd_l = nc.tensor.matmul(
                        R["PRD"], PW2[:, k * 128:(k + 1) * 128],
                        relu[:, k * bl:(k + 1) * bl],
                        start=(k == 0), stop=(k == KC - 1),
                        skip_group_check=True,
                    )
                    if prd_f is None:
                        prd_f = prd_l
                pe_chain(prd_f, prd_l)

                pred = work.tile([1, bl], f16, tag="pred")
                nc.scalar.activation(out=pred, in_=R["PRD"][0:1, :], func=Tanh,
                                     bias=PB2[:, :], scale=SC)
                nc.vector.tensor_mul(
                    S2[0:1, u * bl:(u + 1) * bl], pred,
                    MB[0:1, u * bl:(u + 1) * bl],
                )

                emit_w_region(4, R["G_z"], h_cur, has_gi=True, bias_col=1)
                emit_w_region(8, R["G_n"], h_cur, has_gi=False, bias_col=2)
                gin_f = nc.tensor.matmul(
                    R["GIN"], BC[:, 4 * 128:5 * 128], E4,
                    start=True, stop=False, skip_group_check=True)
                pe_chain(gin_f, gin_f)
                emit_gi(0, R["G_r"], gi_rhs)
                emit_gi(1, R["G_z"], gi_rhs)
                emit_gi(2, R["GIN"], gi_rhs)

                # --- gate math (consumers emitted right after producers so
                # their semaphore posts are not delayed behind later ops) ---
                r_sb = work.tile([128, KC * bl], f16, tag="r_sb")
                nc.scalar.activation(out=r_sb, in_=R["G_r"], func=Sigmoid,
                                     scale=SC)
                u_n = work.tile([128, KC * bl], f16, tag="u_n")
                nc.vector.tensor_mul(u_n, r_sb, R["G_n"])
                z_sb = work.tile([128, KC * bl], f16, tag="z_sb")
                nc.scalar.activation(out=z_sb, in_=R["G_z"], func=Sigmoid,
                                     scale=SC)
                pren = work.tile([128, KC * bl], f32, tag="pren")
                nc.vector.tensor_add(pren, u_n, R["GIN"])
                n_sb = work.tile([128, KC * bl], f16, tag="n_sb")
                nc.scalar.activation(out=n_sb, in_=pren, func=Tanh,
                                     scale=SC)

                # h' = z*h - (z-1)*n ;  t1 = z*h on gpsimd (off the DVE FIFO)
                t1 = work.tile([128, KC * bl], f16, tag="t1")
                nc.gpsimd.tensor_mul(t1, z_sb, h_cur)
                t2 = work.tile([128, KC * bl], f16, tag="t2")
                nc.vector.scalar_tensor_tensor(
                    out=t2, in0=z_sb, scalar=1.0, in1=n_sb,
                    op0=mybir.AluOpType.subtract, op1=mybir.AluOpType.mult,
                )
                nc.vector.tensor_sub(h_new, t1, t2)

                # stream h' out:  outl[t, p, j, b]
                dst = d_out[bass.ds(t_dyn, 1)].rearrange("o p j b -> (o p) j b")
                nc.sync.dma_start(
                    out=dst, in_=h_new.rearrange("p (j b) -> p j b", b=bl)
                )

            n_blocks = t_steps // u_steps
            with tc.For_i(
                0, n_blocks, 1, hint_engines=(mybir.EngineType.PE,)
            ) as iv:
                S2 = io.tile([128, u_steps * bl], f16, tag="S2")
                nc.vector.memset(S2, 0.0)
                MB = io.tile([1, u_steps * bl], f16, tag="MB")
                nc.sync.dma_start(
                    out=S2[1:2, :].rearrange("p (u b) -> p u b", b=bl),
                    in_=d_a[bass.ds(iv * u_steps, u_steps)].unsqueeze(0),
                )
                nc.sync.dma_start(
                    out=MB[0:1, :].rearrange("p (u b) -> p u b", b=bl),
                    in_=d_m[bass.ds(iv * u_steps, u_steps)].unsqueeze(0),
                )
                for u in range(u_steps):
                    step(u, S2, MB, iv * u_steps + u)

    nc.compile()
    return nc


def _prep_core_inputs(inputs, core, t_steps=T, bl=BL):
    """Build the per-core input map (numpy) for core id `core`."""
    f16 = np.float16
    direction = 0 if core < 4 else 1  # 0 fwd, 1 bwd
    bg = core % 4
    sl = slice(bg * bl, (bg + 1) * bl)

    x = np.asarray(inputs["x"], np.float32)[:, :, 0]      # [B, T]
    msk = np.asarray(inputs["mask"]).astype(np.float32)[:, :, 0]
    pfx = "wf" if direction == 0 else "wb"
    w_ih = np.asarray(inputs[f"{pfx}_ih"], np.float32)[:, 0]   # [3H]
    w_hh = np.asarray(inputs[f"{pfx}_hh"], np.float32)         # [3H, H]
    b_ih = np.asarray(inputs[f"b{pfx[1]}_ih"], np.float32)
    b_hh = np.asarray(inputs[f"b{pfx[1]}_hh"], np.float32)
    p_w1 = np.asarray(inputs["p_w1"], np.float32)
    p_b1 = np.asarray(inputs["p_b1"], np.float32)
    p_w2 = np.asarray(inputs["p_w2"], np.float32)
    p_b2 = np.asarray(inputs["p_b2"], np.float32)

    xs = x[sl].T.copy()      # [T, bl]
    ms = msk[sl].T.copy()
    if direction == 1:
        xs = xs[::-1].copy()
        ms = ms[::-1].copy()
    a_arr = (xs * (1.0 - ms)).astype(f16)
    m_arr = ms.astype(f16)

    W = np.concatenate([w_hh, p_w1], axis=0) * WSCALE    # [2048, 512]
    Wr = W.reshape(MC, 128, KC, 128)                     # [m, c, k, p]
    wt = Wr.transpose(3, 0, 2, 1).reshape(128, MC * KC * 128).astype(f16)

    # gi stationaries: [128,128] blocks, rows 0,1 = w_ih chunk (rest zero);
    # contract with S2 whose rows are [tmp; a; 0...].
    gil = np.zeros((128, 12 * 128), np.float32)
    gil[0] = gil[1] = w_ih * WSCALE
    gil = gil.astype(f16)

    bias_regions = [
        b_ih[0:H] + b_hh[0:H],          # r
        b_ih[H:2 * H] + b_hh[H:2 * H],  # z
        b_hh[2 * H:3 * H],              # n: b_hh only
        p_b1,                           # PH
        b_ih[2 * H:3 * H],              # GIN: b_ih_n
    ]
    bc = np.zeros((128, 5 * 128), np.float32)
    bc[:KC] = np.concatenate(
        [br.reshape(KC, 128) for br in bias_regions], axis=1) * WSCALE
    bc = bc.astype(f16)

    e4 = np.zeros((128, KC * bl), np.float32)
    for j in range(KC):
        e4[j, j * bl:(j + 1) * bl] = 1.0

    pw2 = np.zeros((128, KC * 128), np.float32)
    for k in range(KC):
        pw2[:, k * 128] = p_w2[0][k * 128:(k + 1) * 128]

    return {
        "wt": wt, "gil": gil, "bc": bc,
        "e4": e4.astype(f16), "pw2t": pw2.astype(f16),
        "pb2": p_b2.reshape(1, 1).astype(np.float32),
        "a_arr": a_arr[:t_steps], "m_arr": m_arr[:t_steps],
    }


def _assemble(results, t_steps=T, bl=BL):
    """results: list of 8 per-core dicts with 'outl' [T,128,KC,bl] fp16."""
    out = np.zeros((B, t_steps, 2 * H), np.float32)
    for core in range(NCORES):
        direction = 0 if core < 4 else 1
        bg = core % 4
        arr = np.asarray(results[core]["outl"], np.float16).astype(np.float32)
        # [t, p, j, b] -> [b, t, j, p] -> [b, t, 512]
        arr = arr.transpose(3, 0, 2, 1).reshape(bl, t_steps, H)
        if direction == 1:
            arr = arr[:, ::-1]
        out[bg * bl:(bg + 1) * bl, :, direction * H:(direction + 1) * H] = arr
    return out


def kernel(**inputs):
    from concourse.bass_utils import run_bass_kernel_spmd

    key = (T, U_DEF, BL)
    if key not in _cache:
        _cache[key] = _build_program(T, U_DEF, BL)
    nc = _cache[key]

    in_maps = [_prep_core_inputs(inputs, c) for c in range(NCORES)]
    res = run_bass_kernel_spmd(
        nc, in_maps, core_ids=list(range(NCORES)), trace=False
    )
    return _assemble(res.results)


# revision 18
# speedup vs baseline: 1.0063x; 1.0063x over previous
# kernel.py — Bidirectional masked-GRU-with-predictor on 8 Trainium2 NeuronCores.
#
# Problem (reference.py): B=128, T=1024, H=512
#   per step, per direction:
#     x_in = where(mask, predictor(h), x)            predictor: Linear(H,H)->ReLU->Linear(H,1)->Tanh
#     h    = GRUCell(h, x_in)                        PyTorch gate order (r, z, n)
#   output [B, T, 2H] = concat(fwd hidden states, time-reversed bwd hidden states)
#
# Sharding: 8 cores = 2 directions x 4 batch groups of 32.  All cores run the
# SAME Bass program; per-core data differs (bwd cores get time-reversed x/mask
# and their outputs are flipped back on the host).
#
# On-core layout ("feature-major, chunk-in-free"):
#   h^T kept as [128 partitions = feature%128, (j,b)] where j = feature//128
#   (4 chunks), b = local batch (32).  Stationary = W^T 128x128 blocks.
#   ALL stationaries are zero-padded to [128,128] so the PE's background
#   weight loading never re-configures (mixed tile shapes cost ~100-160ns per
#   switch).  PSUM: one bank per live accumulation region (start=True clears
#   has_written for the WHOLE bank; PE-write + DVE/ACT-read of one bank is a
#   fatal collision), double-buffered by step parity: G_r, G_z, G_n, and a
#   shared PH->PRD->GIN bank, x2 = all 8 banks.  fp16 matmuls + fp32 PSUM.

import numpy as np

B, T, H = 128, 1024, 512
NCORES = 8
BL = B // 4          # 32: batch per core (4 groups x 2 directions)
KC = H // 128        # 4 contraction chunks
MC = (3 * H + H) // 128  # 16 output chunks (w_hh 12 + p_w1 4)
U_DEF = 32           # time steps per For_i iteration
WSCALE = 256.0       # stationary pre-scale (exact power of two; exact in fp16)

_cache = {}


def _build_program(t_steps=T, u_steps=U_DEF, bl=BL, n_cores=NCORES):
    import concourse.bacc as bacc
    import concourse.bass as bass
    import concourse.tile as tile
    from concourse.tile import add_dep_helper
    from concourse import mybir

    f16 = mybir.dt.float16
    f32 = mybir.dt.float32

    nc = bacc.Bacc(
        "TRN2",
        target_bir_lowering=False,
        debug=False,
        enable_asserts=False,
        num_devices=n_cores,
    )

    # ---- DRAM tensors (per-core data; same names on every core) ----
    d_wt = nc.dram_tensor("wt", [128, MC * KC * 128], f16,
                          kind="ExternalInput").ap()
    d_gi = nc.dram_tensor("gil", [128, 12 * 128], f16,
                          kind="ExternalInput").ap()
    d_bc = nc.dram_tensor("bc", [128, 5 * 128], f16,
                          kind="ExternalInput").ap()
    d_a = nc.dram_tensor("a_arr", [t_steps, bl], f16,
                         kind="ExternalInput").ap()
    d_m = nc.dram_tensor("m_arr", [t_steps, bl], f16,
                         kind="ExternalInput").ap()
    d_e4 = nc.dram_tensor("e4", [128, KC * bl], f16, kind="ExternalInput").ap()
    d_pw2 = nc.dram_tensor("pw2t", [128, KC * 128], f16,
                           kind="ExternalInput").ap()
    d_pb2 = nc.dram_tensor("pb2", [1, 1], f32, kind="ExternalInput").ap()
    d_out = nc.dram_tensor("outl", [t_steps, 128, KC, bl], f16,
                           kind="ExternalOutput").ap()

    Tanh = mybir.ActivationFunctionType.Tanh
    Sigmoid = mybir.ActivationFunctionType.Sigmoid
    SC = 1.0 / WSCALE

    with tile.TileContext(nc) as tc:
        import contextlib

        with contextlib.ExitStack() as ctx:
            consts = ctx.enter_context(tc.tile_pool(name="consts", bufs=1))
            psum = ctx.enter_context(tc.tile_pool(name="psum", bufs=1, space="PSUM"))
            work = ctx.enter_context(tc.tile_pool(name="work", bufs=2))
            io = ctx.enter_context(tc.tile_pool(name="io", bufs=2))

            # ---- constant preload ----
            WT = consts.tile([128, MC * KC * 128], f16, tag="WT")
            GIL = consts.tile([128, 12 * 128], f16, tag="GIL")
            BC = consts.tile([128, 5 * 128], f16, tag="BC")
            E4 = consts.tile([128, KC * bl], f16, tag="E4")
            PW2 = consts.tile([128, KC * 128], f16, tag="PW2")
            PB2 = consts.tile([1, 1], f32, tag="PB2")
            for dst, src in (
                (WT, d_wt), (GIL, d_gi), (BC, d_bc),
                (E4, d_e4), (PW2, d_pw2), (PB2, d_pb2),
            ):
                nc.sync.dma_start(out=dst, in_=src)

            # persistent ping-pong hidden state, fp16, [128, (j,b)]
            h0 = consts.tile([128, KC * bl], f16, tag="h0")
            h1 = consts.tile([128, KC * bl], f16, tag="h1")
            nc.vector.memset(h0, 0.0)
            nc.vector.memset(h1, 0.0)
            h_tiles = [h0, h1]

            # PSUM: G_r, G_z, G_n own a bank; PH -> PRD -> GIN share the 4th
            # (serial within a step); x2 step parity = 8 banks.
            W_ = KC * bl
            def mk_psum(tagp):
                g_r = psum.tile([128, W_], f32, tag=f"G_r{tagp}",
                                name=f"G_r{tagp}")
                g_z = psum.tile([128, W_], f32, tag=f"G_z{tagp}",
                                name=f"G_z{tagp}")
                g_n = psum.tile([128, W_], f32, tag=f"G_n{tagp}",
                                name=f"G_n{tagp}")
                phb = psum.tile([128, 2 * W_ + bl], f32, tag=f"PHB{tagp}",
                                name=f"PHB{tagp}")
                return {
                    "G_r": g_r, "G_z": g_z, "G_n": g_n,
                    "PH": phb[:, 0:W_],
                    "GIN": phb[:, W_:2 * W_],
                    "PRD": phb[:, 2 * W_:2 * W_ + bl],
                }
            P = [mk_psum(f"{p}") for p in range(2)]

            def w_block(m, k):
                bi = m * KC + k
                return WT[:, bi * 128:(bi + 1) * 128]

            state = {"prev": None}

            def pe_chain(first, last):
                if state["prev"] is not None and first is not None:
                    add_dep_helper(first.ins, state["prev"].ins, sync=False)
                if last is not None:
                    state["prev"] = last

            def emit_w_region(base_m, region, h_cur, has_gi, bias_col):
                # One E4 bias matmul opens the region (the only start=True for
                # its bank this step); W matmuls accumulate.
                first = nc.tensor.matmul(
                    region, BC[:, bias_col * 128:(bias_col + 1) * 128],
                    E4, start=True, stop=False, skip_group_check=True,
                )
                last = first
                for j in range(KC):
                    m = base_m + j
                    for k in range(KC):
                        last = nc.tensor.matmul(
                            region[:, j * bl:(j + 1) * bl],
                            w_block(m, k),
                            h_cur[:, k * bl:(k + 1) * bl],
                            start=False,
                            stop=(not has_gi and k == KC - 1),
                            skip_group_check=True,
                        )
                pe_chain(first, last)

            def emit_gi(g_idx, region, gi_rhs):
                first = last = None
                for j in range(KC):
                    gj = g_idx * KC + j
                    last = nc.tensor.matmul(
                        region[:, j * bl:(j + 1) * bl],
                        GIL[:, gj * 128:(gj + 1) * 128],
                        gi_rhs,
                        start=False, stop=True, skip_group_check=True,
                    )
                    if first is None:
                        first = last
                pe_chain(first, last)

            def step(u, S2, MB, t_dyn):
                h_cur = h_tiles[u % 2]
                h_new = h_tiles[(u + 1) % 2]
                R = P[u % 2]
                gi_rhs = S2[:, u * bl:(u + 1) * bl]

                # --- PE order: PH, W_r, PRD, W_z, gi_r, gi_z, W_n, GIN ---
                emit_w_region(12, R["PH"], h_cur, has_gi=False, bias_col=3)

                relu = work.tile([128, KC * bl], f16, tag="relu")
                nc.vector.tensor_scalar_max(relu, R["PH"], 0.0)

                emit_w_region(0, R["G_r"], h_cur, has_gi=True, bias_col=0)

                prd_f = prd_l = None
                for k in range(KC):
                    prd_l = nc.tensor.matmul(
                        R["PRD"], PW2[:, k * 128:(k + 1) * 128],
                        relu[:, k * bl:(k + 1) * bl],
                        start=(k == 0), stop=(k == KC - 1),
                        skip_group_check=True,
                    )
                    if prd_f is None:
                        prd_f = prd_l
                pe_chain(prd_f, prd_l)

                pred = work.tile([1, bl], f16, tag="pred")
                nc.scalar.activation(out=pred, in_=R["PRD"][0:1, :], func=Tanh,
                                     bias=PB2[:, :], scale=SC)
                nc.vector.tensor_mul(
                    S2[0:1, u * bl:(u + 1) * bl], pred,
                    MB[0:1, u * bl:(u + 1) * bl],
                )

                emit_w_region(4, R["G_z"], h_cur, has_gi=True, bias_col=1)
                emit_w_region(8, R["G_n"], h_cur, has_gi=False, bias_col=2)
                emit_gi(0, R["G_r"], gi_rhs)
                emit_gi(1, R["G_z"], gi_rhs)
                # GIN opener stays after the gi's: its start=True write to the
                # shared PH/PRD bank must follow pred's PRD read anyway.
                gin_f = nc.tensor.matmul(
                    R["GIN"], BC[:, 4 * 128:5 * 128], E4,
                    start=True, stop=False, skip_group_check=True)
                pe_chain(gin_f, gin_f)
                emit_gi(2, R["GIN"], gi_rhs)

                # --- gate math (consumers emitted right after producers so
                # their semaphore posts are not delayed behind later ops) ---
                r_sb = work.tile([128, KC * bl], f16, tag="r_sb")
                nc.scalar.activation(out=r_sb, in_=R["G_r"], func=Sigmoid,
                                     scale=SC)
                u_n = work.tile([128, KC * bl], f16, tag="u_n")
                nc.vector.tensor_mul(u_n, r_sb, R["G_n"])
                z_sb = work.tile([128, KC * bl], f16, tag="z_sb")
                nc.scalar.activation(out=z_sb, in_=R["G_z"], func=Sigmoid,
                                     scale=SC)
                pren = work.tile([128, KC * bl], f32, tag="pren")
                nc.vector.tensor_add(pren, u_n, R["GIN"])
                n_sb = work.tile([128, KC * bl], f16, tag="n_sb")
                nc.scalar.activation(out=n_sb, in_=pren, func=Tanh,
                                     scale=SC)

                # h' = z*h - (z-1)*n ;  t1 = z*h on gpsimd (off the DVE FIFO)
                t1 = work.tile([128, KC * bl], f16, tag="t1")
                nc.gpsimd.tensor_mul(t1, z_sb, h_cur)
                t2 = work.tile([128, KC * bl], f16, tag="t2")
                nc.vector.scalar_tensor_tensor(
                    out=t2, in0=z_sb, scalar=1.0, in1=n_sb,
                    op0=mybir.AluOpType.subtract, op1=mybir.AluOpType.mult,
                )
                nc.vector.tensor_sub(h_new, t1, t2)

                # stream h' out:  outl[t, p, j, b]
                dst = d_out[bass.ds(t_dyn, 1)].rearrange("o p j b -> (o p) j b")
                nc.sync.dma_start(
                    out=dst, in_=h_new.rearrange("p (j b) -> p j b", b=bl)
                )

            n_blocks = t_steps // u_steps
            with tc.For_i(
                0, n_blocks, 1, hint_engines=(mybir.EngineType.PE,)
            ) as iv:
                S2 = io.tile([128, u_steps * bl], f16, tag="S2")
                nc.vector.memset(S2, 0.0)
                MB = io.tile([1, u_steps * bl], f16, tag="MB")
                nc.sync.dma_start(
                    out=S2[1:2, :].rearrange("p (u b) -> p u b", b=bl),
                    in_=d_a[bass.ds(iv * u_steps, u_steps)].unsqueeze(0),
                )
                nc.sync.dma_start(
                    out=MB[0:1, :].rearrange("p (u b) -> p u b", b=bl),
                    in_=d_m[bass.ds(iv * u_steps, u_steps)].unsqueeze(0),
                )
                for u in range(u_steps):
                    step(u, S2, MB, iv * u_steps + u)

    nc.compile()
    return nc


def _prep_core_inputs(inputs, core, t_steps=T, bl=BL):
    """Build the per-core input map (numpy) for core id `core`."""
    f16 = np.float16
    direction = 0 if core < 4 else 1  # 0 fwd, 1 bwd
    bg = core % 4
    sl = slice(bg * bl, (bg + 1) * bl)

    x = np.asarray(inputs["x"], np.float32)[:, :, 0]      # [B, T]
    msk = np.asarray(inputs["mask"]).astype(np.float32)[:, :, 0]
    pfx = "wf" if direction == 0 else "wb"
    w_ih = np.asarray(inputs[f"{pfx}_ih"], np.float32)[:, 0]   # [3H]
    w_hh = np.asarray(inputs[f"{pfx}_hh"], np.float32)         # [3H, H]
    b_ih = np.asarray(inputs[f"b{pfx[1]}_ih"], np.float32)
    b_hh = np.asarray(inputs[f"b{pfx[1]}_hh"], np.float32)
    p_w1 = np.asarray(inputs["p_w1"], np.float32)
    p_b1 = np.asarray(inputs["p_b1"], np.float32)
    p_w2 = np.asarray(inputs["p_w2"], np.float32)
    p_b2 = np.asarray(inputs["p_b2"], np.float32)

    xs = x[sl].T.copy()      # [T, bl]
    ms = msk[sl].T.copy()
    if direction == 1:
        xs = xs[::-1].copy()
        ms = ms[::-1].copy()
    a_arr = (xs * (1.0 - ms)).astype(f16)
    m_arr = ms.astype(f16)

    W = np.concatenate([w_hh, p_w1], axis=0) * WSCALE    # [2048, 512]
    Wr = W.reshape(MC, 128, KC, 128)                     # [m, c, k, p]
    wt = Wr.transpose(3, 0, 2, 1).reshape(128, MC * KC * 128).astype(f16)

    # gi stationaries: [128,128] blocks, rows 0,1 = w_ih chunk (rest zero);
    # contract with S2 whose rows are [tmp; a; 0...].
    gil = np.zeros((128, 12 * 128), np.float32)
    gil[0] = gil[1] = w_ih * WSCALE
    gil = gil.astype(f16)

    bias_regions = [
        b_ih[0:H] + b_hh[0:H],          # r
        b_ih[H:2 * H] + b_hh[H:2 * H],  # z
        b_hh[2 * H:3 * H],              # n: b_hh only
        p_b1,                           # PH
        b_ih[2 * H:3 * H],              # GIN: b_ih_n
    ]
    bc = np.zeros((128, 5 * 128), np.float32)
    bc[:KC] = np.concatenate(
        [br.reshape(KC, 128) for br in bias_regions], axis=1) * WSCALE
    bc = bc.astype(f16)

    e4 = np.zeros((128, KC * bl), np.float32)
    for j in range(KC):
        e4[j, j * bl:(j + 1) * bl] = 1.0

    pw2 = np.zeros((128, KC * 128), np.float32)
    for k in range(KC):
        pw2[:, k * 128] = p_w2[0][k * 128:(k + 1) * 128]

    return {
        "wt": wt, "gil": gil, "bc": bc,
        "e4": e4.astype(f16), "pw2t": pw2.astype(f16),
        "pb2": p_b2.reshape(1, 1).astype(np.float32),
        "a_arr": a_arr[:t_steps], "m_arr": m_arr[:t_steps],
    }


def _assemble(results, t_steps=T, bl=BL):
    """results: list of 8 per-core dicts with 'outl' [T,128,KC,bl] fp16."""
    out = np.zeros((B, t_steps, 2 * H), np.float32)
    for core in range(NCORES):
        direction = 0 if core < 4 else 1
        bg = core % 4
        arr = np.asarray(results[core]["outl"], np.float16).astype(np.float32)
        # [t, p, j, b] -> [b, t, j, p] -> [b, t, 512]
        arr = arr.transpose(3, 0, 2, 1).reshape(bl, t_steps, H)
        if direction == 1:
            arr = arr[:, ::-1]
        out[bg * bl:(bg + 1) * bl, :, direction * H:(direction + 1) * H] = arr
    return out


def kernel(**inputs):
    from concourse.bass_utils import run_bass_kernel_spmd

    key = (T, U_DEF, BL)
    if key not in _cache:
        _cache[key] = _build_program(T, U_DEF, BL)
    nc = _cache[key]

    in_maps = [_prep_core_inputs(inputs, c) for c in range(NCORES)]
    res = run_bass_kernel_spmd(
        nc, in_maps, core_ids=list(range(NCORES)), trace=False
    )
    return _assemble(res.results)


# revision 19
# speedup vs baseline: 1.0286x; 1.0222x over previous
# kernel.py — Bidirectional masked-GRU-with-predictor on 8 Trainium2 NeuronCores.
#
# Problem (reference.py): B=128, T=1024, H=512
#   per step, per direction:
#     x_in = where(mask, predictor(h), x)            predictor: Linear(H,H)->ReLU->Linear(H,1)->Tanh
#     h    = GRUCell(h, x_in)                        PyTorch gate order (r, z, n)
#   output [B, T, 2H] = concat(fwd hidden states, time-reversed bwd hidden states)
#
# Sharding: 8 cores = 2 directions x 4 batch groups of 32.  All cores run the
# SAME Bass program; per-core data differs (bwd cores get time-reversed x/mask
# and their outputs are flipped back on the host).
#
# On-core layout ("feature-major, chunk-in-free"):
#   h^T kept as [128 partitions = feature%128, (j,b)] where j = feature//128
#   (4 chunks), b = local batch (32).  Stationary = W^T 128x128 blocks.
#   ALL stationaries are zero-padded to [128,128] so the PE's background
#   weight loading never re-configures (mixed tile shapes cost ~100-160ns per
#   switch).  PSUM: one bank per live accumulation region (start=True clears
#   has_written for the WHOLE bank; PE-write + DVE/ACT-read of one bank is a
#   fatal collision), double-buffered by step parity: G_r, G_z, G_n, and a
#   shared PH->PRD->GIN bank, x2 = all 8 banks.  fp16 matmuls + fp32 PSUM.

import numpy as np

B, T, H = 128, 1024, 512
NCORES = 8
BL = B // 4          # 32: batch per core (4 groups x 2 directions)
KC = H // 128        # 4 contraction chunks
MC = (3 * H + H) // 128  # 16 output chunks (w_hh 12 + p_w1 4)
U_DEF = 32           # time steps per For_i iteration
WSCALE = 256.0       # stationary pre-scale (exact power of two; exact in fp16)

_cache = {}


def _build_program(t_steps=T, u_steps=U_DEF, bl=BL, n_cores=NCORES):
    import concourse.bacc as bacc
    import concourse.bass as bass
    import concourse.tile as tile
    from concourse.tile import add_dep_helper
    from concourse import mybir

    f16 = mybir.dt.float16
    f32 = mybir.dt.float32

    nc = bacc.Bacc(
        "TRN2",
        target_bir_lowering=False,
        debug=False,
        enable_asserts=False,
        num_devices=n_cores,
    )

    # ---- DRAM tensors (per-core data; same names on every core) ----
    d_wt = nc.dram_tensor("wt", [128, MC * KC * 128], f16,
                          kind="ExternalInput").ap()
    d_gi = nc.dram_tensor("gil", [128, 12 * 128], f16,
                          kind="ExternalInput").ap()
    d_bc = nc.dram_tensor("bc", [128, 5 * 128], f16,
                          kind="ExternalInput").ap()
    d_a = nc.dram_tensor("a_arr", [t_steps, bl], f16,
                         kind="ExternalInput").ap()
    d_m = nc.dram_tensor("m_arr", [t_steps, bl], f16,
                         kind="ExternalInput").ap()
    d_e4 = nc.dram_tensor("e4", [128, KC * bl], f16, kind="ExternalInput").ap()
    d_pw2 = nc.dram_tensor("pw2t", [128, KC * 128], f16,
                           kind="ExternalInput").ap()
    d_pb2 = nc.dram_tensor("pb2", [1, 1], f32, kind="ExternalInput").ap()
    d_out = nc.dram_tensor("outl", [t_steps, 128, KC, bl], f16,
                           kind="ExternalOutput").ap()

    Tanh = mybir.ActivationFunctionType.Tanh
    Sigmoid = mybir.ActivationFunctionType.Sigmoid
    SC = 1.0 / WSCALE

    with tile.TileContext(nc) as tc:
        import contextlib

        with contextlib.ExitStack() as ctx:
            consts = ctx.enter_context(tc.tile_pool(name="consts", bufs=1))
            psum = ctx.enter_context(tc.tile_pool(name="psum", bufs=1, space="PSUM"))
            work = ctx.enter_context(tc.tile_pool(name="work", bufs=2))
            io = ctx.enter_context(tc.tile_pool(name="io", bufs=2))

            # ---- constant preload ----
            WT = consts.tile([128, MC * KC * 128], f16, tag="WT")
            GIL = consts.tile([128, 12 * 128], f16, tag="GIL")
            BC = consts.tile([128, 5 * 128], f16, tag="BC")
            E4 = consts.tile([128, KC * bl], f16, tag="E4")
            PW2 = consts.tile([128, KC * 128], f16, tag="PW2")
            PB2 = consts.tile([1, 1], f32, tag="PB2")
            for dst, src in (
                (WT, d_wt), (GIL, d_gi), (BC, d_bc),
                (E4, d_e4), (PW2, d_pw2), (PB2, d_pb2),
            ):
                nc.sync.dma_start(out=dst, in_=src)

            # persistent ping-pong hidden state, fp16, [128, (j,b)]
            h0 = consts.tile([128, KC * bl], f16, tag="h0")
            h1 = consts.tile([128, KC * bl], f16, tag="h1")
            nc.vector.memset(h0, 0.0)
            nc.vector.memset(h1, 0.0)
            h_tiles = [h0, h1]

            # PSUM: G_r, G_z, G_n own a bank; PH -> PRD -> GIN share the 4th
            # (serial within a step); x2 step parity = 8 banks.
            W_ = KC * bl
            def mk_psum(tagp):
                g_r = psum.tile([128, W_], f32, tag=f"G_r{tagp}",
                                name=f"G_r{tagp}")
                g_z = psum.tile([128, W_], f32, tag=f"G_z{tagp}",
                                name=f"G_z{tagp}")
                g_n = psum.tile([128, W_], f32, tag=f"G_n{tagp}",
                                name=f"G_n{tagp}")
                phb = psum.tile([128, 2 * W_ + bl], f32, tag=f"PHB{tagp}",
                                name=f"PHB{tagp}")
                return {
                    "G_r": g_r, "G_z": g_z, "G_n": g_n,
                    "PH": phb[:, 0:W_],
                    "GIN": phb[:, W_:2 * W_],
                    "PRD": phb[:, 2 * W_:2 * W_ + bl],
                }
            P = [mk_psum(f"{p}") for p in range(2)]

            def w_block(m, k):
                bi = m * KC + k
                return WT[:, bi * 128:(bi + 1) * 128]

            state = {"prev": None}

            def pe_chain(first, last):
                if state["prev"] is not None and first is not None:
                    add_dep_helper(first.ins, state["prev"].ins, sync=False)
                if last is not None:
                    state["prev"] = last

            def emit_w_region(base_m, region, h_cur, has_gi, bias_col):
                # One E4 bias matmul opens the region (the only start=True for
                # its bank this step); W matmuls accumulate.
                first = nc.tensor.matmul(
                    region, BC[:, bias_col * 128:(bias_col + 1) * 128],
                    E4, start=True, stop=False, skip_group_check=True,
                )
                last = first
                for j in range(KC):
                    m = base_m + j
                    for k in range(KC):
                        last = nc.tensor.matmul(
                            region[:, j * bl:(j + 1) * bl],
                            w_block(m, k),
                            h_cur[:, k * bl:(k + 1) * bl],
                            start=False,
                            stop=(not has_gi and k == KC - 1),
                            skip_group_check=True,
                        )
                pe_chain(first, last)

            def emit_gi(g_idx, region, gi_rhs):
                first = last = None
                for j in range(KC):
                    gj = g_idx * KC + j
                    last = nc.tensor.matmul(
                        region[:, j * bl:(j + 1) * bl],
                        GIL[:, gj * 128:(gj + 1) * 128],
                        gi_rhs,
                        start=False, stop=True, skip_group_check=True,
                    )
                    if first is None:
                        first = last
                pe_chain(first, last)

            def step(u, S2, MB, t_dyn):
                h_cur = h_tiles[u % 2]
                h_new = h_tiles[(u + 1) % 2]
                R = P[u % 2]
                gi_rhs = S2[:, u * bl:(u + 1) * bl]

                # --- PE order: PH, W_r, PRD, W_z, gi_r, gi_z, W_n, GIN ---
                emit_w_region(12, R["PH"], h_cur, has_gi=False, bias_col=3)

                relu = work.tile([128, KC * bl], f16, tag="relu")
                nc.vector.tensor_scalar_max(relu, R["PH"], 0.0)

                emit_w_region(0, R["G_r"], h_cur, has_gi=True, bias_col=0)

                prd_f = prd_l = None
                for k in range(KC):
                    prd_l = nc.tensor.matmul(
                        R["PRD"], PW2[:, k * 128:(k + 1) * 128],
                        relu[:, k * bl:(k + 1) * bl],
                        start=(k == 0), stop=(k == KC - 1),
                        skip_group_check=True,
                    )
                    if prd_f is None:
                        prd_f = prd_l
                pe_chain(prd_f, prd_l)

                pred = work.tile([1, bl], f16, tag="pred")
                nc.scalar.activation(out=pred, in_=R["PRD"][0:1, :], func=Tanh,
                                     bias=PB2[:, :], scale=SC)
                nc.vector.tensor_mul(
                    S2[0:1, u * bl:(u + 1) * bl], pred,
                    MB[0:1, u * bl:(u + 1) * bl],
                )

                emit_w_region(4, R["G_z"], h_cur, has_gi=True, bias_col=1)
                emit_gi(0, R["G_r"], gi_rhs)
                emit_gi(1, R["G_z"], gi_rhs)
                emit_w_region(8, R["G_n"], h_cur, has_gi=False, bias_col=2)

                gin_f = nc.tensor.matmul(
                    R["GIN"], BC[:, 4 * 128:5 * 128], E4,
                    start=True, stop=False, skip_group_check=True)
                pe_chain(gin_f, gin_f)
                emit_gi(2, R["GIN"], gi_rhs)

                # --- gate math (ACT order: pred, r, z, n) ---
                r_sb = work.tile([128, KC * bl], f16, tag="r_sb")
                nc.scalar.activation(out=r_sb, in_=R["G_r"], func=Sigmoid,
                                     scale=SC)
                z_sb = work.tile([128, KC * bl], f16, tag="z_sb")
                nc.scalar.activation(out=z_sb, in_=R["G_z"], func=Sigmoid,
                                     scale=SC)

                u_n = work.tile([128, KC * bl], f16, tag="u_n")
                nc.vector.tensor_mul(u_n, r_sb, R["G_n"])
                pren = work.tile([128, KC * bl], f32, tag="pren")
                nc.vector.tensor_add(pren, u_n, R["GIN"])
                n_sb = work.tile([128, KC * bl], f16, tag="n_sb")
                nc.scalar.activation(out=n_sb, in_=pren, func=Tanh,
                                     scale=SC)

                # h' = z*h - (z-1)*n ;  t1 = z*h starts right after z_sb
                t1 = work.tile([128, KC * bl], f16, tag="t1")
                nc.vector.tensor_mul(t1, z_sb, h_cur)
                t2 = work.tile([128, KC * bl], f16, tag="t2")
                nc.vector.scalar_tensor_tensor(
                    out=t2, in0=z_sb, scalar=1.0, in1=n_sb,
                    op0=mybir.AluOpType.subtract, op1=mybir.AluOpType.mult,
                )
                nc.vector.tensor_sub(h_new, t1, t2)

                # stream h' out:  outl[t, p, j, b]
                dst = d_out[bass.ds(t_dyn, 1)].rearrange("o p j b -> (o p) j b")
                nc.sync.dma_start(
                    out=dst, in_=h_new.rearrange("p (j b) -> p j b", b=bl)
                )

            n_blocks = t_steps // u_steps
            with tc.For_i(
                0, n_blocks, 1, hint_engines=(mybir.EngineType.PE,)
            ) as iv:
                S2 = io.tile([128, u_steps * bl], f16, tag="S2")
                nc.vector.memset(S2, 0.0)
                MB = io.tile([1, u_steps * bl], f16, tag="MB")
                nc.sync.dma_start(
                    out=S2[1:2, :].rearrange("p (u b) -> p u b", b=bl),
                    in_=d_a[bass.ds(iv * u_steps, u_steps)].unsqueeze(0),
                )
                nc.sync.dma_start(
                    out=MB[0:1, :].rearrange("p (u b) -> p u b", b=bl),
                    in_=d_m[bass.ds(iv * u_steps, u_steps)].unsqueeze(0),
                )
                for u in range(u_steps):
                    step(u, S2, MB, iv * u_steps + u)

    nc.compile()
    return nc


def _prep_core_inputs(inputs, core, t_steps=T, bl=BL):
    """Build the per-core input map (numpy) for core id `core`."""
    f16 = np.float16
    direction = 0 if core < 4 else 1  # 0 fwd, 1 bwd
    bg = core % 4
    sl = slice(bg * bl, (bg + 1) * bl)

    x = np.asarray(inputs["x"], np.float32)[:, :, 0]      # [B, T]
    msk = np.asarray(inputs["mask"]).astype(np.float32)[:, :, 0]
    pfx = "wf" if direction == 0 else "wb"
    w_ih = np.asarray(inputs[f"{pfx}_ih"], np.float32)[:, 0]   # [3H]
    w_hh = np.asarray(inputs[f"{pfx}_hh"], np.float32)         # [3H, H]
    b_ih = np.asarray(inputs[f"b{pfx[1]}_ih"], np.float32)
    b_hh = np.asarray(inputs[f"b{pfx[1]}_hh"], np.float32)
    p_w1 = np.asarray(inputs["p_w1"], np.float32)
    p_b1 = np.asarray(inputs["p_b1"], np.float32)
    p_w2 = np.asarray(inputs["p_w2"], np.float32)
    p_b2 = np.asarray(inputs["p_b2"], np.float32)

    xs = x[sl].T.copy()      # [T, bl]
    ms = msk[sl].T.copy()
    if direction == 1:
        xs = xs[::-1].copy()
        ms = ms[::-1].copy()
    a_arr = (xs * (1.0 - ms)).astype(f16)
    m_arr = ms.astype(f16)

    W = np.concatenate([w_hh, p_w1], axis=0) * WSCALE    # [2048, 512]
    Wr = W.reshape(MC, 128, KC, 128)                     # [m, c, k, p]
    wt = Wr.transpose(3, 0, 2, 1).reshape(128, MC * KC * 128).astype(f16)

    # gi stationaries: [128,128] blocks, rows 0,1 = w_ih chunk (rest zero);
    # contract with S2 whose rows are [tmp; a; 0...].
    gil = np.zeros((128, 12 * 128), np.float32)
    gil[0] = gil[1] = w_ih * WSCALE
    gil = gil.astype(f16)

    bias_regions = [
        b_ih[0:H] + b_hh[0:H],          # r
        b_ih[H:2 * H] + b_hh[H:2 * H],  # z
        b_hh[2 * H:3 * H],              # n: b_hh only
        p_b1,                           # PH
        b_ih[2 * H:3 * H],              # GIN: b_ih_n
    ]
    bc = np.zeros((128, 5 * 128), np.float32)
    bc[:KC] = np.concatenate(
        [br.reshape(KC, 128) for br in bias_regions], axis=1) * WSCALE
    bc = bc.astype(f16)

    e4 = np.zeros((128, KC * bl), np.float32)
    for j in range(KC):
        e4[j, j * bl:(j + 1) * bl] = 1.0

    pw2 = np.zeros((128, KC * 128), np.float32)
    for k in range(KC):
        pw2[:, k * 128] = p_w2[0][k * 128:(k + 1) * 128]

    return {
        "wt": wt, "gil": gil, "bc": bc,
        "e4": e4.astype(f16), "pw2t": pw2.astype(f16),
        "pb2": p_b2.reshape(1, 1).astype(np.float32),
        "a_arr": a_arr[:t_steps], "m_arr": m_arr[:t_steps],
    }


def _assemble(results, t_steps=T, bl=BL):
    """results: list of 8 per-core dicts with 'outl' [T,128,KC,bl] fp16."""
    out = np.zeros((B, t_steps, 2 * H), np.float32)
    for core in range(NCORES):
        direction = 0 if core < 4 else 1
        bg = core % 4
        arr = np.asarray(results[core]["outl"], np.float16).astype(np.float32)
        # [t, p, j, b] -> [b, t, j, p] -> [b, t, 512]
        arr = arr.transpose(3, 0, 2, 1).reshape(bl, t_steps, H)
        if direction == 1:
            arr = arr[:, ::-1]
        out[bg * bl:(bg + 1) * bl, :, direction * H:(direction + 1) * H] = arr
    return out


def kernel(**inputs):
    from concourse.bass_utils import run_bass_kernel_spmd

    key = (T, U_DEF, BL)
    if key not in _cache:
        _cache[key] = _build_program(T, U_DEF, BL)
    nc = _cache[key]

    in_maps = [_prep_core_inputs(inputs, c) for c in range(NCORES)]
    res = run_bass_kernel_spmd(
        nc, in_maps, core_ids=list(range(NCORES)), trace=False
    )
    return _assemble(res.results)
